# revision 18
# baseline (speedup 1.0000x reference)
# Trainium2 Bass kernel for kornia-style ConvSoftArgmax3d (3x3x3 window,
# stride 1, pad 1, temperature 1, output_value=True).
#
# Math: for each (b,c) sample with volume v = x[b,c] (D,H,W):
#   e      = exp(v)                     (global-max shift cancels in ratios)
#   den    = sumpool3x3x3(e)            (zero-padded)
#   coords_z = d + sumpool(dz * e)/den  (dz = relative z offset in window)
#   coords_x = w + sumpool(dw * e)/den
#   coords_y = h + sumpool(dh * e)/den
#   vals     = sumpool(v * e)/den
# All pools share the denominator; eps=1e-8 of the reference is negligible
# (den >= ~0.1 without the max shift) and is skipped.
#
# Layout per plane (H,W)=(256,256): partition p holds rows h=2p and h=2p+1
# ("interleaved H"), free dims (j=2, w=256).  The H-axis 3-tap and the
# D-axis 3-tap run on the tensor engine as banded matmuls accumulated in
# PSUM; the W-axis 3-tap runs on the vector engine as shifted adds over
# zero-padded tiles.
#
# Sharding: 32 independent (b,c) samples -> 4 samples per core x 8 cores.

import numpy as np

_CACHE = {}

D, H, W = 16, 256, 256
SPC = 4          # samples per core
NCORES = 8
P = 128          # partitions
J = 2            # H interleave factor (H = 2*P)
WP = W + 2       # padded width
RING = 5         # per-d field ring depth


def _np_consts():
    """Banded lhsT matrices ([K,M] layout, out = lhsT.T @ rhs) + iota tiles."""
    M1 = np.zeros((P, P), np.float32)   # out_j0 sum taps from in_j1
    M2 = np.zeros((P, P), np.float32)   # out_j1 sum taps from in_j0
    M1Y = np.zeros((P, P), np.float32)  # out_j0 rel-h taps from in_j1
    M2Y = np.zeros((P, P), np.float32)  # out_j1 rel-h taps from in_j0
    for m in range(P):
        # out row h=2m (j0): in rows h=2k+1 -> k in {m-1, m}
        if m - 1 >= 0:
            M1[m - 1, m] = 1.0
            M1Y[m - 1, m] = -1.0   # h_in - h_out = (2m-1) - 2m
        M1[m, m] = 1.0
        M1Y[m, m] = 1.0            # (2m+1) - 2m
        # out row h=2m+1 (j1): in rows h=2k -> k in {m, m+1}
        M2[m, m] = 1.0
        M2Y[m, m] = -1.0           # 2m - (2m+1)
        if m + 1 < P:
            M2[m + 1, m] = 1.0
            M2Y[m + 1, m] = 1.0    # (2m+2) - (2m+1)
    I = np.eye(P, dtype=np.float32)
    iota_w = np.broadcast_to(
        np.arange(W, dtype=np.float32)[None, None, :], (P, J, W)
    ).copy()
    iota_h0 = (2.0 * np.arange(P, dtype=np.float32))[:, None].copy()
    iota_h1 = iota_h0 + 1.0
    dvals = np.broadcast_to(np.arange(D, dtype=np.float32)[None, :],
                            (P, D)).copy()
    return dict(M1=M1, M2=M2, I=I, M1n=-M1, M2n=-M2, In=-I,
                M1Y=M1Y, M2Y=M2Y, iota_w=iota_w,
                iota_h0=iota_h0, iota_h1=iota_h1, dvals=dvals)


def build_nc():
    import concourse.bacc as bacc
    import concourse.tile as tile
    from concourse import mybir

    f32 = mybir.dt.float32
    f32r = mybir.dt.float32r
    Alu = mybir.AluOpType
    Act = mybir.ActivationFunctionType

    nc = bacc.Bacc("TRN2", target_bir_lowering=False)

    import os
    dbg = bool(os.environ.get("KDBG"))

    xs = nc.dram_tensor("xs", [SPC, D, H, W], f32, kind="ExternalInput")
    coords = nc.dram_tensor("coords", [SPC, 3, D, H, W], f32,
                            kind="ExternalOutput")
    vals = nc.dram_tensor("vals", [SPC, D, H, W], f32, kind="ExternalOutput")
    dbg_t = (nc.dram_tensor("dbg", [4, P, J, W], f32, kind="ExternalOutput")
             if dbg else None)

    cn = _np_consts()
    cdram = {k: nc.inline_tensor(v, name=f"c_{k}") for k, v in cn.items()}

    with tile.TileContext(nc) as tc:
        with (
            tc.tile_pool(name="consts", bufs=1) as cpool,
            tc.tile_pool(name="fields", bufs=1) as fpool,
            tc.tile_pool(name="work", bufs=3) as wpool,
            tc.tile_pool(name="outs", bufs=3) as opool,
            tc.tile_pool(name="psum", bufs=1, space="PSUM") as ppool,
        ):
            # ---- load constants into SBUF (banded matrices as fp32r so the
            # tensor engine streams them at full rate; their entries are
            # small integers, exact in fp32r)
            MATS = ("M1", "M2", "I", "M1n", "M2n", "In", "M1Y", "M2Y")
            ct = {}
            for k, v in cn.items():
                dt_k = f32r if k in MATS else f32
                t = cpool.tile([P] + list(v.shape[1:]), dt_k, tag=f"c_{k}")
                nc.sync.dma_start(out=t[:], in_=cdram[k][:].bitcast(dt_k))
                ct[k] = t

            def mat(k):
                return ct[k][:]

            def h_apply(psum_t, rhs_t, sign, first, last):
                """One d-slice contribution of the H-axis banded 3-tap sum:
                j0 out gets (M1 @ rhs_j1) + (I @ rhs_j0), j1 out gets
                (M2 @ rhs_j0) + (I @ rhs_j1).  start=True clears the WHOLE
                PSUM bank, so it goes only on the chronologically first
                matmul into this tile; has_written bits handle per-region
                first-write-overwrite for the rest."""
                m1, m2, ii = (("M1", "M2", "I") if sign > 0
                              else ("M1n", "M2n", "In"))
                rj0 = rhs_t[:, 0, :]
                rj1 = rhs_t[:, 1, :]
                nc.tensor.matmul(psum_t[:, 0, :], mat(m1), rj1,
                                 start=first, stop=False)
                nc.tensor.matmul(psum_t[:, 0, :], mat(ii), rj0,
                                 start=False, stop=False)
                nc.tensor.matmul(psum_t[:, 1, :], mat(m2), rj0,
                                 start=False, stop=False)
                nc.tensor.matmul(psum_t[:, 1, :], mat(ii), rj1,
                                 start=False, stop=last)

            def emit_output(ew, de, pw, s, do):
                dds = [dd for dd in (do - 1, do, do + 1) if 0 <= dd < D]

                den = ppool.tile([P, J, W], f32, tag="den")
                nz = ppool.tile([P, J, W], f32, tag="nz")
                nx = ppool.tile([P, J, W], f32, tag="nx")
                nv = ppool.tile([P, J, W], f32, tag="nv")
                ny = ppool.tile([P, J, W], f32, tag="ny")

                last = len(dds) - 1
                for i, dd in enumerate(dds):
                    h_apply(den, ew[dd], +1, i == 0, i == last)
                for i, dd in enumerate(dds):
                    h_apply(nx, de[dd], +1, i == 0, i == last)
                for i, dd in enumerate(dds):
                    h_apply(nv, pw[dd], +1, i == 0, i == last)
                # z numerator: relative z weights (+1 / -1 on the d+-1 slices)
                zdds = [dd for dd in (do - 1, do + 1) if 0 <= dd < D]
                for i, dd in enumerate(zdds):
                    h_apply(nz, ew[dd], 1 if dd > do else -1,
                            i == 0, i == len(zdds) - 1)
                # y numerator: relative h weights (diagonal term vanishes)
                for i, dd in enumerate(dds):
                    nc.tensor.matmul(ny[:, 0, :], mat("M1Y"),
                                     ew[dd][:, 1, :],
                                     start=i == 0, stop=False)
                    nc.tensor.matmul(ny[:, 1, :], mat("M2Y"),
                                     ew[dd][:, 0, :],
                                     start=False, stop=i == last)

                rt = opool.tile([P, J, W], f32, tag="r")
                nc.vector.reciprocal_approx_fast(out=rt[:], in_=den[:])

                if dbg_t is not None and s == 0 and do == 7:
                    for di, src in enumerate((den, nz, ny, nv)):
                        dtap = opool.tile([P, J, W], f32, tag="dtap")
                        nc.vector.tensor_copy(out=dtap[:], in_=src[:])
                        nc.sync.dma_start(out=dbg_t[di], in_=dtap[:])

                vt = opool.tile([P, J, W], f32, tag="v")
                nc.vector.tensor_tensor(vt[:], nv[:], rt[:], op=Alu.mult)

                czt = opool.tile([P, J, W], f32, tag="cz")
                nc.vector.tensor_tensor(czt[:], nz[:], rt[:], op=Alu.mult)
                nc.scalar.activation(czt[:], czt[:], Act.Identity,
                                     bias=ct["dvals"][:, do:do + 1])

                cyt = opool.tile([P, J, W], f32, tag="cy")
                nc.vector.tensor_tensor(cyt[:], ny[:], rt[:], op=Alu.mult)
                nc.scalar.activation(cyt[:, 0, :], cyt[:, 0, :], Act.Identity,
                                     bias=ct["iota_h0"][:])
                nc.scalar.activation(cyt[:, 1, :], cyt[:, 1, :], Act.Identity,
                                     bias=ct["iota_h1"][:])

                cxt = opool.tile([P, J, W], f32, tag="cx")
                nc.vector.tensor_tensor(cxt[:], nx[:], rt[:], op=Alu.mult)
                nc.vector.tensor_tensor(cxt[:], cxt[:], ct["iota_w"][:],
                                        op=Alu.add)

                rr = "(p j) w -> p j w"
                nc.sync.dma_start(out=coords[s, 0, do].rearrange(rr, j=J),
                                  in_=czt[:])
                nc.sync.dma_start(out=coords[s, 1, do].rearrange(rr, j=J),
                                  in_=cxt[:])
                nc.sync.dma_start(out=coords[s, 2, do].rearrange(rr, j=J),
                                  in_=cyt[:])
                nc.sync.dma_start(out=vals[s, do].rearrange(rr, j=J),
                                  in_=vt[:])

            for s in range(SPC):
                ew, de, pw = {}, {}, {}
                for d in range(D):
                    # ---- input phase for plane d
                    xt = wpool.tile([P, J, W], f32, tag="x")
                    nc.sync.dma_start(
                        out=xt[:],
                        in_=xs[s, d].rearrange("(p j) w -> p j w", j=J),
                    )
                    et = wpool.tile([P, J, WP], f32, tag="epad")
                    pt = wpool.tile([P, J, WP], f32, tag="ppad")
                    # zero the pad columns (0 and WP-1 of each j section)
                    nc.vector.memset(et[:, :, 0], 0.0)
                    nc.vector.memset(et[:, :, WP - 1], 0.0)
                    nc.vector.memset(pt[:, :, 0], 0.0)
                    nc.vector.memset(pt[:, :, WP - 1], 0.0)
                    nc.scalar.activation(et[:, :, 1:W + 1], xt[:], Act.Exp)
                    nc.vector.tensor_tensor(pt[:, :, 1:W + 1],
                                            et[:, :, 1:W + 1], xt[:],
                                            op=Alu.mult)

                    ewt = fpool.tile([P, J, W], f32r, tag=f"ew{d % RING}")
                    det = fpool.tile([P, J, W], f32r, tag=f"de{d % RING}")
                    pwt = fpool.tile([P, J, W], f32r, tag=f"pw{d % RING}")
                    # W-axis 3-tap sums / diff (zero-padded)
                    nc.vector.tensor_tensor(ewt[:], et[:, :, 0:W],
                                            et[:, :, 1:W + 1], op=Alu.add)
                    nc.vector.tensor_tensor(ewt[:], ewt[:],
                                            et[:, :, 2:W + 2], op=Alu.add)
                    nc.vector.tensor_tensor(det[:], et[:, :, 2:W + 2],
                                            et[:, :, 0:W], op=Alu.subtract)
                    nc.vector.tensor_tensor(pwt[:], pt[:, :, 0:W],
                                            pt[:, :, 1:W + 1], op=Alu.add)
                    nc.vector.tensor_tensor(pwt[:], pwt[:],
                                            pt[:, :, 2:W + 2], op=Alu.add)
                    ew[d], de[d], pw[d] = ewt, det, pwt

                    # ---- output phase for plane do = d-1 (window complete)
                    if d >= 1:
                        emit_output(ew, de, pw, s, d - 1)
                emit_output(ew, de, pw, s, D - 1)
    nc.compile()
    return nc


def _get_nc():
    if "nc" not in _CACHE:
        _CACHE["nc"] = build_nc()
    return _CACHE["nc"]


def _make_runner():
    """Build a jitted 8-core shard_map executor for the bass program.
    Mirrors concourse.bass2jax.run_bass_via_pjrt but reusable/timeable."""
    import jax
    import numpy as np_
    from jax.sharding import Mesh, PartitionSpec
    from jax.experimental.shard_map import shard_map
    from concourse import bass2jax, mybir

    bass2jax.install_neuronx_cc_hook()
    nc = _get_nc()

    pname = nc.partition_id_tensor.name if nc.partition_id_tensor else None
    in_names, out_names, out_avals, zero_shapes = [], [], [], []
    for alloc in nc.m.functions[0].allocations:
        if not isinstance(alloc, mybir.MemoryLocationSet):
            continue
        name = alloc.memorylocations[0].name
        if alloc.kind == "ExternalInput":
            if name != pname:
                in_names.append(name)
        elif alloc.kind == "ExternalOutput":
            shape = tuple(alloc.tensor_shape)
            dtype = mybir.dt.np(alloc.dtype)
            out_names.append(name)
            out_avals.append(jax.core.ShapedArray(shape, dtype))
            zero_shapes.append((shape, dtype))
    n_params = len(in_names)
    all_in_names = in_names + out_names
    if pname is not None:
        all_in_names = all_in_names + [pname]

    def _body(*args):
        operands = list(args)
        if pname is not None:
            operands.append(bass2jax.partition_id_tensor())
        outs = bass2jax._bass_exec_p.bind(
            *operands,
            out_avals=tuple(out_avals),
            in_names=tuple(all_in_names),
            out_names=tuple(out_names),
            lowering_input_output_aliases=(),
            sim_require_finite=True,
            sim_require_nnan=True,
            nc=nc,
        )
        return tuple(outs)

    devices = jax.devices()[:NCORES]
    mesh = Mesh(np.asarray(devices), ("core",))
    nio = n_params + len(out_names)
    sharded = jax.jit(
        shard_map(_body, mesh=mesh,
                  in_specs=(PartitionSpec("core"),) * nio,
                  out_specs=(PartitionSpec("core"),) * len(out_names)),
        donate_argnums=tuple(range(n_params, nio)),
        keep_unused=True,
    )

    def make_zeros():
        return [np_.zeros((NCORES * s[0], *s[1:]), dt)
                for s, dt in zero_shapes]

    return sharded, make_zeros, out_names


def get_runner():
    if "runner" not in _CACHE:
        _CACHE["runner"] = _make_runner()
    return _CACHE["runner"]


def _unpack(out_arrs, out_names):
    B, C = 4, 8
    d = {name: np.asarray(a) for name, a in zip(out_names, out_arrs)}
    coords = d["coords"].reshape(B, C, 3, D, H, W)
    valsr = d["vals"].reshape(B, C, D, H, W)
    return coords, valsr


def run(x, **kwargs):
    sharded, make_zeros, out_names = get_runner()
    xs = np.ascontiguousarray(x, dtype=np.float32).reshape(
        NCORES * SPC, D, H, W)
    out_arrs = sharded(xs, *make_zeros())
    return _unpack(out_arrs, out_names), None


def kernel(x):
    out, _ = run(x)
    return out


# revision 24
# speedup vs baseline: 115.8125x; 115.8125x over previous
# Trainium2 Bass kernel for kornia-style ConvSoftArgmax3d (3x3x3 window,
# stride 1, pad 1, temperature 1, output_value=True).
#
# Math: for each (b,c) sample with volume v = x[b,c] (D,H,W):
#   e      = exp(v)                     (global-max shift cancels in ratios)
#   den    = sumpool3x3x3(e)            (zero-padded)
#   coords_z = d + sumpool(dz * e)/den  (dz = relative z offset in window)
#   coords_x = w + sumpool(dw * e)/den
#   coords_y = h + sumpool(dh * e)/den
#   vals     = sumpool(v * e)/den
# All pools share the denominator; eps=1e-8 of the reference is negligible
# (den >= ~0.1 without the max shift) and is skipped.
#
# Layout per plane (H,W)=(256,256): partition p holds rows h=2p and h=2p+1
# ("interleaved H"), free dims (j=2, w=256).  The H-axis 3-tap and the
# D-axis 3-tap run on the tensor engine as banded matmuls accumulated in
# PSUM; the W-axis 3-tap runs on the vector engine as shifted adds over
# zero-padded tiles.
#
# Sharding: 32 independent (b,c) samples -> 4 samples per core x 8 cores.

import numpy as np

_CACHE = {}

D, H, W = 16, 256, 256
SPC = 4          # samples per core
NCORES = 8
P = 128          # partitions
J = 2            # H interleave factor (H = 2*P)
WP = W + 2       # padded width
RING = 5         # per-d field ring depth


def _np_consts():
    """Banded lhsT matrices ([K,M] layout, out = lhsT.T @ rhs) + iota tiles."""
    M1 = np.zeros((P, P), np.float32)   # out_j0 sum taps from in_j1
    M2 = np.zeros((P, P), np.float32)   # out_j1 sum taps from in_j0
    M1Y = np.zeros((P, P), np.float32)  # out_j0 rel-h taps from in_j1
    M2Y = np.zeros((P, P), np.float32)  # out_j1 rel-h taps from in_j0
    for m in range(P):
        # out row h=2m (j0): in rows h=2k+1 -> k in {m-1, m}
        if m - 1 >= 0:
            M1[m - 1, m] = 1.0
            M1Y[m - 1, m] = -1.0   # h_in - h_out = (2m-1) - 2m
        M1[m, m] = 1.0
        M1Y[m, m] = 1.0            # (2m+1) - 2m
        # out row h=2m+1 (j1): in rows h=2k -> k in {m, m+1}
        M2[m, m] = 1.0
        M2Y[m, m] = -1.0           # 2m - (2m+1)
        if m + 1 < P:
            M2[m + 1, m] = 1.0
            M2Y[m + 1, m] = 1.0    # (2m+2) - (2m+1)
    I = np.eye(P, dtype=np.float32)
    iota_w = np.broadcast_to(
        np.arange(W, dtype=np.float32)[None, None, :], (P, J, W)
    ).copy()
    iota_h0 = (2.0 * np.arange(P, dtype=np.float32))[:, None].copy()
    iota_h1 = iota_h0 + 1.0
    dvals = np.broadcast_to(np.arange(D, dtype=np.float32)[None, :],
                            (P, D)).copy()
    return dict(M1=M1, M2=M2, I=I, M1n=-M1, M2n=-M2, In=-I,
                M1Y=M1Y, M2Y=M2Y, iota_w=iota_w,
                iota_h0=iota_h0, iota_h1=iota_h1, dvals=dvals)


def build_nc():
    import concourse.bacc as bacc
    import concourse.tile as tile
    from concourse import mybir

    f32 = mybir.dt.float32
    f32r = mybir.dt.float32r
    Alu = mybir.AluOpType
    Act = mybir.ActivationFunctionType

    nc = bacc.Bacc("TRN2", target_bir_lowering=False)

    import os
    dbg = bool(os.environ.get("KDBG"))

    xs = nc.dram_tensor("xs", [SPC, D, H, W], f32, kind="ExternalInput")
    coords = nc.dram_tensor("coords", [SPC, 3, D, H, W], f32,
                            kind="ExternalOutput")
    vals = nc.dram_tensor("vals", [SPC, D, H, W], f32, kind="ExternalOutput")
    dbg_t = (nc.dram_tensor("dbg", [4, P, J, W], f32, kind="ExternalOutput")
             if dbg else None)

    cn = _np_consts()
    cdram = {k: nc.inline_tensor(v, name=f"c_{k}") for k, v in cn.items()}

    with tile.TileContext(nc) as tc:
        with (
            tc.tile_pool(name="consts", bufs=1) as cpool,
            tc.tile_pool(name="fields", bufs=1) as fpool,
            tc.tile_pool(name="pads", bufs=1) as padpool,
            tc.tile_pool(name="work", bufs=3) as wpool,
            tc.tile_pool(name="outs", bufs=3) as opool,
            tc.tile_pool(name="psum", bufs=1, space="PSUM") as ppool,
        ):
            # ---- load constants into SBUF (banded matrices as fp32r so the
            # tensor engine streams them at full rate; their entries are
            # small integers, exact in fp32r)
            MATS = ("M1", "M2", "I", "M1n", "M2n", "In", "M1Y", "M2Y")
            ct = {}
            for k, v in cn.items():
                dt_k = f32r if k in MATS else f32
                t = cpool.tile([P] + list(v.shape[1:]), dt_k, tag=f"c_{k}")
                nc.sync.dma_start(out=t[:], in_=cdram[k][:].bitcast(dt_k))
                ct[k] = t

            def mat(k):
                return ct[k][:]

            def h_apply(psum_t, rhs_t, sign, first, last):
                """One d-slice contribution of the H-axis banded 3-tap sum:
                j0 out gets (M1 @ rhs_j1) + (I @ rhs_j0), j1 out gets
                (M2 @ rhs_j0) + (I @ rhs_j1).  start=True clears the WHOLE
                PSUM bank, so it goes only on the chronologically first
                matmul into this tile; has_written bits handle per-region
                first-write-overwrite for the rest."""
                m1, m2, ii = (("M1", "M2", "I") if sign > 0
                              else ("M1n", "M2n", "In"))
                rj0 = rhs_t[:, 0, :]
                rj1 = rhs_t[:, 1, :]
                nc.tensor.matmul(psum_t[:, 0, :], mat(m1), rj1,
                                 start=first, stop=False)
                nc.tensor.matmul(psum_t[:, 0, :], mat(ii), rj0,
                                 start=False, stop=False)
                nc.tensor.matmul(psum_t[:, 1, :], mat(m2), rj0,
                                 start=False, stop=False)
                nc.tensor.matmul(psum_t[:, 1, :], mat(ii), rj1,
                                 start=False, stop=last)

            def emit_output(ew, de, pw, s, do):
                dds = [dd for dd in (do - 1, do, do + 1) if 0 <= dd < D]

                den = ppool.tile([P, J, W], f32, tag="den")
                nz = ppool.tile([P, J, W], f32, tag="nz")
                nx = ppool.tile([P, J, W], f32, tag="nx")
                nv = ppool.tile([P, J, W], f32, tag="nv")
                ny = ppool.tile([P, J, W], f32, tag="ny")

                last = len(dds) - 1
                for i, dd in enumerate(dds):
                    h_apply(den, ew[dd], +1, i == 0, i == last)
                for i, dd in enumerate(dds):
                    h_apply(nx, de[dd], +1, i == 0, i == last)
                for i, dd in enumerate(dds):
                    h_apply(nv, pw[dd], +1, i == 0, i == last)
                # z numerator: relative z weights (+1 / -1 on the d+-1 slices)
                zdds = [dd for dd in (do - 1, do + 1) if 0 <= dd < D]
                for i, dd in enumerate(zdds):
                    h_apply(nz, ew[dd], 1 if dd > do else -1,
                            i == 0, i == len(zdds) - 1)
                # y numerator: relative h weights (diagonal term vanishes)
                for i, dd in enumerate(dds):
                    nc.tensor.matmul(ny[:, 0, :], mat("M1Y"),
                                     ew[dd][:, 1, :],
                                     start=i == 0, stop=False)
                    nc.tensor.matmul(ny[:, 1, :], mat("M2Y"),
                                     ew[dd][:, 0, :],
                                     start=False, stop=i == last)

                rt = opool.tile([P, J, W], f32, tag="r")
                nc.vector.reciprocal_approx_fast(out=rt[:], in_=den[:])

                if dbg_t is not None and s == 0 and do == 7:
                    for di, src in enumerate((den, nz, ny, nv)):
                        dtap = opool.tile([P, J, W], f32, tag="dtap")
                        nc.vector.tensor_copy(out=dtap[:], in_=src[:])
                        nc.sync.dma_start(out=dbg_t[di], in_=dtap[:])

                vt = opool.tile([P, J, W], f32, tag="v")
                nc.vector.tensor_tensor(vt[:], nv[:], rt[:], op=Alu.mult)

                czt = opool.tile([P, J, W], f32, tag="cz")
                nc.vector.tensor_tensor(czt[:], nz[:], rt[:], op=Alu.mult)
                nc.scalar.activation(czt[:], czt[:], Act.Identity,
                                     bias=ct["dvals"][:, do:do + 1])

                cyt = opool.tile([P, J, W], f32, tag="cy")
                nc.vector.tensor_tensor(cyt[:], ny[:], rt[:], op=Alu.mult)
                nc.scalar.activation(cyt[:, 0, :], cyt[:, 0, :], Act.Identity,
                                     bias=ct["iota_h0"][:])
                nc.scalar.activation(cyt[:, 1, :], cyt[:, 1, :], Act.Identity,
                                     bias=ct["iota_h1"][:])

                cxt = opool.tile([P, J, W], f32, tag="cx")
                nc.vector.tensor_tensor(cxt[:], nx[:], rt[:], op=Alu.mult)
                nc.gpsimd.tensor_tensor(cxt[:], cxt[:], ct["iota_w"][:],
                                        op=Alu.add)

                rr = "(p j) w -> p j w"
                nc.sync.dma_start(out=coords[s, 0, do].rearrange(rr, j=J),
                                  in_=czt[:])
                nc.sync.dma_start(out=coords[s, 1, do].rearrange(rr, j=J),
                                  in_=cxt[:])
                nc.sync.dma_start(out=coords[s, 2, do].rearrange(rr, j=J),
                                  in_=cyt[:])
                nc.sync.dma_start(out=vals[s, do].rearrange(rr, j=J),
                                  in_=vt[:])

            # padded e / p ring tiles, hoisted so the pad columns are zeroed
            # exactly once (per-plane writes never touch them)
            NPAD = 3
            ep_ring = [padpool.tile([P, J, WP], f32, tag=f"ep{i}",
                                    name=f"ep{i}") for i in range(NPAD)]
            pp_ring = [padpool.tile([P, J, WP], f32, tag=f"pp{i}",
                                    name=f"pp{i}") for i in range(NPAD)]
            for t in ep_ring + pp_ring:
                nc.gpsimd.memset(t[:, :, 0], 0.0)
                nc.gpsimd.memset(t[:, :, WP - 1], 0.0)

            for s in range(SPC):
                ew, de, pw = {}, {}, {}
                for d in range(D):
                    # ---- input phase for plane d
                    xt = wpool.tile([P, J, W], f32, tag="x")
                    nc.sync.dma_start(
                        out=xt[:],
                        in_=xs[s, d].rearrange("(p j) w -> p j w", j=J),
                    )
                    et = ep_ring[(s * D + d) % NPAD]
                    pt = pp_ring[(s * D + d) % NPAD]
                    nc.scalar.activation(et[:, :, 1:W + 1], xt[:], Act.Exp)
                    nc.gpsimd.tensor_tensor(pt[:, :, 1:W + 1],
                                            et[:, :, 1:W + 1], xt[:],
                                            op=Alu.mult)

                    ewt = fpool.tile([P, J, W], f32r, tag=f"ew{d % RING}")
                    det = fpool.tile([P, J, W], f32r, tag=f"de{d % RING}")
                    pwt = fpool.tile([P, J, W], f32r, tag=f"pw{d % RING}")
                    # W-axis 3-tap sums / diff (zero-padded)
                    nc.vector.tensor_tensor(ewt[:], et[:, :, 0:W],
                                            et[:, :, 1:W + 1], op=Alu.add)
                    nc.vector.tensor_tensor(ewt[:], ewt[:],
                                            et[:, :, 2:W + 2], op=Alu.add)
                    nc.vector.tensor_tensor(det[:], et[:, :, 2:W + 2],
                                            et[:, :, 0:W], op=Alu.subtract)
                    nc.gpsimd.tensor_tensor(pwt[:], pt[:, :, 0:W],
                                            pt[:, :, 1:W + 1], op=Alu.add)
                    nc.gpsimd.tensor_tensor(pwt[:], pwt[:],
                                            pt[:, :, 2:W + 2], op=Alu.add)
                    ew[d], de[d], pw[d] = ewt, det, pwt

                    # ---- output phase for plane do = d-1 (window complete)
                    if d >= 1:
                        emit_output(ew, de, pw, s, d - 1)
                emit_output(ew, de, pw, s, D - 1)
    nc.compile()
    return nc


def _get_nc():
    if "nc" not in _CACHE:
        _CACHE["nc"] = build_nc()
    return _CACHE["nc"]


def _make_runner():
    """Build a jitted 8-core shard_map executor for the bass program.
    Mirrors concourse.bass2jax.run_bass_via_pjrt but reusable/timeable."""
    import jax
    import numpy as np_
    from jax.sharding import Mesh, PartitionSpec
    from jax.experimental.shard_map import shard_map
    from concourse import bass2jax, mybir

    bass2jax.install_neuronx_cc_hook()
    nc = _get_nc()

    pname = nc.partition_id_tensor.name if nc.partition_id_tensor else None
    in_names, out_names, out_avals, zero_shapes = [], [], [], []
    for alloc in nc.m.functions[0].allocations:
        if not isinstance(alloc, mybir.MemoryLocationSet):
            continue
        name = alloc.memorylocations[0].name
        if alloc.kind == "ExternalInput":
            if name != pname:
                in_names.append(name)
        elif alloc.kind == "ExternalOutput":
            shape = tuple(alloc.tensor_shape)
            dtype = mybir.dt.np(alloc.dtype)
            out_names.append(name)
            out_avals.append(jax.core.ShapedArray(shape, dtype))
            zero_shapes.append((shape, dtype))
    n_params = len(in_names)
    all_in_names = in_names + out_names
    if pname is not None:
        all_in_names = all_in_names + [pname]

    def _body(*args):
        operands = list(args)
        if pname is not None:
            operands.append(bass2jax.partition_id_tensor())
        outs = bass2jax._bass_exec_p.bind(
            *operands,
            out_avals=tuple(out_avals),
            in_names=tuple(all_in_names),
            out_names=tuple(out_names),
            lowering_input_output_aliases=(),
            sim_require_finite=True,
            sim_require_nnan=True,
            nc=nc,
        )
        return tuple(outs)

    devices = jax.devices()[:NCORES]
    mesh = Mesh(np.asarray(devices), ("core",))
    nio = n_params + len(out_names)
    sharded = jax.jit(
        shard_map(_body, mesh=mesh,
                  in_specs=(PartitionSpec("core"),) * nio,
                  out_specs=(PartitionSpec("core"),) * len(out_names)),
        donate_argnums=tuple(range(n_params, nio)),
        keep_unused=True,
    )

    def make_zeros():
        return [np_.zeros((NCORES * s[0], *s[1:]), dt)
                for s, dt in zero_shapes]

    return sharded, make_zeros, out_names


def get_runner():
    if "runner" not in _CACHE:
        _CACHE["runner"] = _make_runner()
    return _CACHE["runner"]


def _unpack(out_arrs, out_names):
    B, C = 4, 8
    d = {name: np.asarray(a) for name, a in zip(out_names, out_arrs)}
    coords = d["coords"].reshape(B, C, 3, D, H, W)
    valsr = d["vals"].reshape(B, C, D, H, W)
    return coords, valsr


def run(x, **kwargs):
    sharded, make_zeros, out_names = get_runner()
    xs = np.ascontiguousarray(x, dtype=np.float32).reshape(
        NCORES * SPC, D, H, W)
    out_arrs = sharded(xs, *make_zeros())
    return _unpack(out_arrs, out_names), None


def kernel(x):
    out, _ = run(x)
    return out
